# revision 19
# baseline (speedup 1.0000x reference)
"""Trainium2 Bass kernel for nn_DisAttention (dense transformer attention
variant with weighted-renormalized softmax and dual output projections).

Sharding: 8 cores = (batch b in 0..3) x (head-half in {0,1}).
Core (b, 0) handles heads 0-5 of batch b and produces mu[b] (channels 0-383
of the attention output are exactly heads 0-5).
Core (b, 1) handles heads 6-11 and produces logsigma[b].
Every core also writes its attn[b, h0:h0+6] slice. No collectives needed.

Math identity used on-device:
  softmax(s)*w / sum(softmax(s)*w) == exp(s + log w) / sum_m exp(s + log w)
so mask and log-weights fold into one additive matrix M' (host-computed),
injected into PSUM via an identity matmul before the QK accumulation; one
Exp activation (accum_out = row sums) finishes the softmax numerator.

Attention output needs [n, m]-layout tiles (for the attn DRAM write) AND
[m, n]-layout tiles (as the moving operand of the P@V matmul). PE-transposing
attn is a trap: transpose-mode matmuls don't count as PE activity for the HAM
clock gate, so the PE drops to 1.2 GHz for the whole phase. Instead the
scores are computed twice - once per layout, by swapping which of q/k is the
stationary operand (the transposed mask M'^T comes from the host) - so the PE
only ever runs real matmuls and stays at 2.4 GHz. P@V consumes the
unnormalized [m, n] exp tiles; the normalization (1/rowsum) is applied to the
64-row P@V output via a ones-matmul broadcast of the reciprocal row.
"""
import numpy as np
from contextlib import ExitStack

import concourse.bass as bass
import concourse.bacc as bacc
import concourse.tile as tile
import concourse.mybir as mybir
from concourse.bass_utils import run_bass_kernel_spmd
from concourse.masks import make_identity

F32 = mybir.dt.float32
F32R = mybir.dt.float32r
BF16 = mybir.dt.bfloat16
AF = mybir.ActivationFunctionType

B, N, C, H = 4, 1024, 768, 12
HPC = H // 2          # heads per core = 6
DH = C // H           # 64
SCALE = float((C * H) ** -0.5)
NT = N // 128         # 8 n-tiles
CT = C // 128         # 6 c-chunks
PD = HPC * DH         # 384 projection contraction dim


def _build_body(ctx, tc, ins, outs):
    nc = tc.nc
    xT_ap, Mp_ap, MpT_ap, wq_ap, wk_ap, wv_ap, wp_ap, pb_ap = ins
    attn_ap, proj_ap = outs

    # pools
    sb1 = ctx.enter_context(tc.tile_pool(name="sb1", bufs=1))     # singles
    sbw = ctx.enter_context(tc.tile_pool(name="sbw", bufs=1))     # weights/masks
    sbig = ctx.enter_context(tc.tile_pool(name="sbig", bufs=6))   # xT + f tiles
    set_ = ctx.enter_context(tc.tile_pool(name="set", bufs=3))    # eT tiles (+wp)
    ssm = ctx.enter_context(tc.tile_pool(name="ssm", bufs=4))     # den/recip/bcast
    sqk = ctx.enter_context(tc.tile_pool(name="sqk", bufs=1))     # qT/kT/v/mu_inT
    sout = ctx.enter_context(tc.tile_pool(name="sout", bufs=2))   # proj out stage
    psc = ctx.enter_context(tc.tile_pool(name="psc", bufs=2, space="PSUM"))   # scores
    pso = ctx.enter_context(tc.tile_pool(name="pso", bufs=1, space="PSUM"))   # outT
    psr = ctx.enter_context(tc.tile_pool(name="psr", bufs=1, space="PSUM"))   # den rows

    # identity / ones constants
    ident32 = sb1.tile([128, 128], F32, tag="ident32")
    make_identity(nc, ident32)
    ident = sb1.tile([128, 128], F32R, tag="ident")
    nc.vector.tensor_copy(ident[:], ident32[:])
    ones32 = sb1.tile([1, 128], F32, tag="ones32")
    nc.vector.memset(ones32[:], 1.0)
    ones = sb1.tile([1, 128], F32R, tag="ones")
    nc.vector.tensor_copy(ones[:], ones32[:])

    # ---- static loads (cast to f32r during DMA on the SWDGE path) ----
    wq_sb = []
    wk_sb = []
    wv_sb = []
    for c in range(CT):
        t = sbw.tile([128, PD], F32R, tag=f"wq{c}", name=f"wq{c}")
        nc.gpsimd.dma_start(out=t[:], in_=wq_ap[c * 128:(c + 1) * 128, :])
        wq_sb.append(t)
        t = sbw.tile([128, PD], F32R, tag=f"wk{c}", name=f"wk{c}")
        nc.gpsimd.dma_start(out=t[:], in_=wk_ap[c * 128:(c + 1) * 128, :])
        wk_sb.append(t)
        t = sbw.tile([128, PD], F32R, tag=f"wv{c}", name=f"wv{c}")
        nc.gpsimd.dma_start(out=t[:], in_=wv_ap[c * 128:(c + 1) * 128, :])
        wv_sb.append(t)
    pb_sb = sb1.tile([1, C], F32R, tag="pb")
    nc.gpsimd.dma_start(out=pb_sb[:], in_=pb_ap[:])
    Mp_sb = []
    MpT_sb = []
    for i in range(NT):
        t = sbw.tile([128, N], F32R, tag=f"mp{i}", name=f"mp{i}")
        nc.gpsimd.dma_start(out=t[:], in_=Mp_ap[i * 128:(i + 1) * 128, :])
        Mp_sb.append(t)
        t = sbw.tile([128, N], F32R, tag=f"mpt{i}", name=f"mpt{i}")
        nc.gpsimd.dma_start(out=t[:], in_=MpT_ap[i * 128:(i + 1) * 128, :])
        MpT_sb.append(t)

    # ---- QKV projections ----
    xT_sb = []
    for c in range(CT):
        t = sbig.tile([128, N], F32R, tag="big", name=f"xT{c}")
        nc.gpsimd.dma_start(out=t[:], in_=xT_ap[c * 128:(c + 1) * 128, :])
        xT_sb.append(t)

    # qT/kT per head-pair: [128, N] with head 2j in rows 0-63, 2j+1 in 64-127
    q_sb = []
    k_sb = []
    for j in range(HPC // 2):
        for which, wsb, dst in (("q", wq_sb, q_sb), ("k", wk_sb, k_sb)):
            ps = psc.tile([128, N], F32, tag="sc", name=f"ps{which}{j}")
            for c in range(CT):
                for nh in range(2):
                    nc.tensor.matmul(
                        ps[:, nh * 512:(nh + 1) * 512],
                        wsb[c][:, j * 128:(j + 1) * 128],
                        xT_sb[c][:, nh * 512:(nh + 1) * 512],
                        start=(c == 0), stop=(c == CT - 1))
            t = sqk.tile([128, N], BF16, tag=f"{which}{j}", name=f"{which}{j}")
            nc.vector.tensor_copy(t[:], ps[:])
            dst.append(t)

    # v: [t-tile 128, PD] natural layout
    v_sb = []
    for i in range(NT):
        ps = psc.tile([128, N], F32, tag="sc", name=f"psv{i}")
        nc.tensor.matmul(ps[:, 0:PD], xT_sb[0][:, i * 128:(i + 1) * 128],
                         wv_sb[0][:], start=True, stop=False)
        for c in range(1, CT):
            nc.tensor.matmul(ps[:, 0:PD], xT_sb[c][:, i * 128:(i + 1) * 128],
                             wv_sb[c][:], start=False, stop=(c == CT - 1))
        t = sqk.tile([128, PD], F32R, tag=f"v{i}", name=f"v{i}")
        nc.vector.tensor_copy(t[:], ps[:, 0:PD])
        v_sb.append(t)

    # ---- attention per head ----
    mu_inT = [sqk.tile([128, N], F32R, tag=f"muin{j}", name=f"muin{j}")
              for j in range(HPC // 2)]
    for h in range(HPC):
        j, odd = divmod(h, 2)
        r0 = odd * 64
        qh = q_sb[j]
        kh = k_sb[j]

        # A-side: [n, m] scores -> exp (+rowsum) -> normalize -> attn DMA
        denblk = ssm.tile([128, NT], F32, tag="denblk", name=f"denblk{h}", bufs=2)
        for i in range(NT):
            ps = psc.tile([128, N], F32, tag="sc", name=f"psA{h}_{i}")
            for nh in range(2):
                sl = slice(nh * 512, (nh + 1) * 512)
                nc.tensor.matmul(ps[:, sl], ident[:], Mp_sb[i][:, sl],
                                 start=True, stop=False)
                nc.tensor.matmul(ps[:, sl],
                                 qh[r0:r0 + 64, i * 128:(i + 1) * 128],
                                 kh[r0:r0 + 64, sl],
                                 start=False, stop=True)
            f = sbig.tile([128, N], F32R, tag="big", name=f"f{h}_{i}")
            nc.scalar.activation(f[:], ps[:], AF.Exp,
                                 accum_out=denblk[:, i:i + 1])
            rec = ssm.tile([128, 1], F32, tag="rec", name=f"rec{h}_{i}")
            nc.vector.reciprocal(rec[:], denblk[:, i:i + 1])
            nc.vector.tensor_scalar_mul(f[:], f[:], rec[:])
            nc.sync.dma_start(out=attn_ap[h, i * 128:(i + 1) * 128, :], in_=f[:])

        # reciprocal of row sums, as a [1, N] row for the outT normalization
        rblk = ssm.tile([128, NT], F32, tag="rblk", name=f"rblk{h}", bufs=2)
        nc.vector.reciprocal(rblk[:], denblk[:])
        rrow_ps = psr.tile([1, N], F32, tag="rrow", name=f"rrow{h}")
        for i in range(NT):
            nc.tensor.transpose(rrow_ps[:, i * 128:(i + 1) * 128],
                                rblk[:, i:i + 1], ident32[:])
        rrow = ssm.tile([1, N], F32R, tag="rrowsb", name=f"rrowsb{h}", bufs=2)
        nc.scalar.copy(rrow[:], rrow_ps[:])

        # T-side: [m, n] scores -> exp -> P@V (unnormalized)
        po = pso.tile([128, N], F32, tag="outT", name=f"outT{h}")
        for mj in range(NT):
            ps = psc.tile([128, N], F32, tag="sc", name=f"psT{h}_{mj}")
            for nh in range(2):
                sl = slice(nh * 512, (nh + 1) * 512)
                nc.tensor.matmul(ps[:, sl], ident[:], MpT_sb[mj][:, sl],
                                 start=True, stop=False)
                nc.tensor.matmul(ps[:, sl],
                                 kh[r0:r0 + 64, mj * 128:(mj + 1) * 128],
                                 qh[r0:r0 + 64, sl],
                                 start=False, stop=True)
            et = set_.tile([128, N], F32R, tag="et", name=f"et{h}_{mj}")
            nc.scalar.activation(et[:], ps[:], AF.Exp)
            for nh in range(2):
                sl = slice(nh * 512, (nh + 1) * 512)
                nc.tensor.matmul(po[0:64, sl],
                                 v_sb[mj][:, h * 64:(h + 1) * 64],
                                 et[:, sl],
                                 start=(mj == 0), stop=(mj == NT - 1))

        # broadcast 1/den over the 64 dh rows and normalize outT
        bc_ps = psc.tile([128, N], F32, tag="sc", name=f"bc{h}")
        for nh in range(2):
            sl = slice(nh * 512, (nh + 1) * 512)
            nc.tensor.matmul(bc_ps[0:64, sl], ones[:, 0:64], rrow[:, sl],
                             start=True, stop=True)
        bc = ssm.tile([64, N], F32, tag="bcsb", name=f"bcsb{h}", bufs=2)
        nc.scalar.copy(bc[:], bc_ps[0:64, :])
        nc.vector.tensor_tensor(
            out=mu_inT[j][r0:r0 + 64, :], in0=po[0:64, :], in1=bc[:],
            op=mybir.AluOpType.mult)

    # ---- output projection: proj[t, :] = mu_inT.T @ wp + bias ----
    wp_sb = []
    for c in range(PD // 128):
        t = set_.tile([128, C], F32R, tag="et", name=f"wp{c}")
        nc.gpsimd.dma_start(out=t[:], in_=wp_ap[c * 128:(c + 1) * 128, :])
        wp_sb.append(t)
    for i in range(NT):
        ps = psc.tile([128, N], F32, tag="sc", name=f"pj{i}")
        for sp in range(2):
            sl = slice(sp * 512, min((sp + 1) * 512, C))
            nc.tensor.matmul(ps[:, sl], ones[:, 0:128], pb_sb[:, sl],
                             start=True, stop=False)
            for c in range(PD // 128):
                nc.tensor.matmul(ps[:, sl],
                                 mu_inT[c][:, i * 128:(i + 1) * 128],
                                 wp_sb[c][:, sl],
                                 start=False, stop=(c == PD // 128 - 1))
        o = sout.tile([128, C], F32, tag="po", name=f"po{i}")
        nc.scalar.copy(o[:], ps[:, 0:C])
        nc.sync.dma_start(out=proj_ap[i * 128:(i + 1) * 128, :], in_=o[:])


_CACHE = {}


def _get_compiled():
    if "nc" in _CACHE:
        return _CACHE["nc"]
    nc = bacc.Bacc("TRN2", target_bir_lowering=False, debug=False, num_devices=8)
    ins = [
        nc.dram_tensor("xT", [C, N], F32, kind="ExternalInput").ap(),
        nc.dram_tensor("Mp", [N, N], F32, kind="ExternalInput").ap(),
        nc.dram_tensor("MpT", [N, N], F32, kind="ExternalInput").ap(),
        nc.dram_tensor("wq", [C, PD], F32, kind="ExternalInput").ap(),
        nc.dram_tensor("wk", [C, PD], F32, kind="ExternalInput").ap(),
        nc.dram_tensor("wv", [C, PD], F32, kind="ExternalInput").ap(),
        nc.dram_tensor("wp", [PD, C], F32, kind="ExternalInput").ap(),
        nc.dram_tensor("pb", [1, C], F32, kind="ExternalInput").ap(),
    ]
    outs = [
        nc.dram_tensor("attn_o", [HPC, N, N], F32R, kind="ExternalOutput").ap(),
        nc.dram_tensor("proj_o", [N, C], F32, kind="ExternalOutput").ap(),
    ]
    with tile.TileContext(nc) as tc:
        with ExitStack() as ctx:
            _build_body(ctx, tc, ins, outs)
    nc.compile()
    _CACHE["nc"] = nc
    return nc


def _in_maps(x, mask, weight, qkv_w, mu_w, mu_b, ls_w, ls_b):
    x = np.asarray(x, dtype=np.float32)
    mask = np.asarray(mask, dtype=np.float32)
    weight = np.asarray(weight, dtype=np.float32)
    qkv_w = np.asarray(qkv_w, dtype=np.float32)
    logw = np.log(np.asarray(weight, dtype=np.float64) + 1e-10).astype(np.float32)
    Wq, Wk, Wv = qkv_w[0:C], qkv_w[C:2 * C], qkv_w[2 * C:3 * C]
    maps = []
    for core in range(8):
        b, half = divmod(core, 2)
        hs = half * PD
        Mp = mask[b, 0] + logw[b][None, :]
        wp = np.ascontiguousarray((mu_w if half == 0 else ls_w).T, dtype=np.float32)
        pb = np.asarray(mu_b if half == 0 else ls_b, dtype=np.float32).reshape(1, C)
        maps.append({
            "xT": np.ascontiguousarray(x[b].T),
            "Mp": np.ascontiguousarray(Mp),
            "MpT": np.ascontiguousarray(Mp.T),
            "wq": np.ascontiguousarray((Wq[hs:hs + PD] * SCALE).T),
            "wk": np.ascontiguousarray(Wk[hs:hs + PD].T),
            "wv": np.ascontiguousarray(Wv[hs:hs + PD].T),
            "wp": wp,
            "pb": pb,
        })
    return maps


def kernel(x, mask, weight, qkv_w, mu_w, mu_b, ls_w, ls_b):
    nc = _get_compiled()
    maps = _in_maps(x, mask, weight, qkv_w, mu_w, mu_b, ls_w, ls_b)
    res = run_bass_kernel_spmd(nc, maps, core_ids=list(range(8)))
    r = res.results
    mu = np.stack([r[2 * b]["proj_o"] for b in range(B)])
    ls = np.stack([r[2 * b + 1]["proj_o"] for b in range(B)])
    attn = np.stack([
        np.concatenate([r[2 * b]["attn_o"], r[2 * b + 1]["attn_o"]], axis=0)
        for b in range(B)
    ])
    return mu, ls, attn
